# revision 12
# baseline (speedup 1.0000x reference)
"""Trainium2 Bass kernel for nn_MemoryBlock (scatter_memory).

Mathematical identity: softmax over the memory-unit axis U produces rows
that sum to exactly 1, so

    out[b] = relu( mean_u( sum_n attn[b,n,u] * V[b,n,:] ) @ Wo + bo )
           = relu( (sum_n X[b,n,:]) @ W2 + c2 )

with W2 = (Wv/U) @ Wo and c2 = (N/U)*bv @ Wo + bo folded on the host --
the whole K/scores/softmax path cancels algebraically, leaving a
memory-bound column-sum of X plus one tiny matmul.

Device-side layout choices (all per core, data-parallel over batch B):
- X is downcast to fp16 on the host (tolerance is loose; colsum error
  ~3e-4), halving HBM traffic to 8.4 MB/core.
- SDMA engine 15 (the known-slow one; serves SBUF partitions 92-95 and
  124-127) gets a half-density row layout: those 8 partitions carry 34
  rows/batch vs 66 on the other 120 partitions, so the stream is never
  gated by the straggler engine.
- DVE folds row pairs (fp16 add) before the PE ones-matmul column-sum,
  halving TensorE streaming time so DMA stays the bottleneck.
- Finale is per-batch (batch 0's output DMA completes mid-run) with a
  minimal engine-hop chain; ACT issues the output DMAs right after its
  own relu.
"""

import contextlib

import numpy as np

B, N, FEAT, MEM, U = 16, 8192, 256, 128, 512
NCORES = 8
BPC = B // NCORES

# row layout: 120 "fast" partitions x 66 rows + 8 "slow" partitions
# (92-95, 124-127 = SDMA engine 15) x 34 rows = 8192 rows
RF = 66            # rows per fast partition per batch
RS = 34            # rows per slow partition per batch
NCH = 9            # chunks: 8 x 8 rows + 1 x 2 rows (fast partitions)
CHR = 8            # rows per full chunk
# slow partitions: chunks 0-3 full (8 rows), chunk 4 has 2 rows, 5-8 none

_built = None


def _ensure_axon_hooks():
    try:
        import antenv.axon_hooks  # noqa: F401
        return
    except ImportError:
        pass
    import sys
    import types

    m = types.ModuleType("antenv.axon_hooks")
    holder = [None]
    m.set_axon_ntff_profile_hook = lambda h: holder.__setitem__(0, h)
    m.get_axon_ntff_profile_hook = lambda: holder[0]
    sys.modules["antenv.axon_hooks"] = m
    try:
        import antenv

        antenv.axon_hooks = m
    except ImportError:
        pass


def _build():
    import concourse.bacc as bacc
    import concourse.mybir as mybir

    f32 = mybir.dt.float32
    f16 = mybir.dt.float16
    AF = mybir.ActivationFunctionType
    ALU = mybir.AluOpType
    nc = bacc.Bacc(None, enable_partition_id=False, monotonic_sem_count=0)

    XA_d = nc.dram_tensor("XA", [BPC, 92, RF * FEAT], f16, kind="ExternalInput")
    XB_d = nc.dram_tensor("XB", [BPC, 2, 4, RS * FEAT], f16, kind="ExternalInput")
    XC_d = nc.dram_tensor("XC", [BPC, 28, RF * FEAT], f16, kind="ExternalInput")
    W2_d = nc.dram_tensor("W2", [2, 128, MEM], f32, kind="ExternalInput")
    zero_d = nc.dram_tensor("zeros", [4, CHR * FEAT], f16, kind="ExternalInput")
    # bias padded to 512B/partition rows for line-rate descriptors: col0=c2
    bias_d = nc.dram_tensor("biasc", [MEM, 128], f32, kind="ExternalInput")
    out_d = nc.dram_tensor("outT", [MEM, BPC], f32, kind="ExternalOutput")

    CW = CHR * FEAT          # 2048 fp16 cols per full chunk
    MW = 2 * FEAT            # 512 cols in the mini chunk (2 rows)

    ctx = contextlib.ExitStack()
    with ctx:
        xts, fts = [], []
        for b in range(BPC):
            for j in range(NCH):
                w = CW if j < 8 else MW
                xts.append(
                    ctx.enter_context(nc.sbuf_tensor(f"xt{b}_{j}", [128, w], f16))
                )
                if j < 8:
                    fts.append(
                        ctx.enter_context(
                            nc.sbuf_tensor(f"ft{b}_{j}", [128, w // 2], f16)
                        )
                    )
                else:
                    fts.append(None)
        ones16 = ctx.enter_context(nc.sbuf_tensor("ones16", [128, 1], f16))
        one_f = ctx.enter_context(nc.sbuf_tensor("one_f", [1, 1], f32))
        w2_sb = ctx.enter_context(nc.sbuf_tensor("w2_sb", [128, 2 * MEM], f32))
        bias_sb = ctx.enter_context(nc.sbuf_tensor("bias_sb", [128, 128], f32))
        srows = [
            ctx.enter_context(nc.sbuf_tensor(f"srow{b}", [1, 2 * FEAT], f32))
            for b in range(BPC)
        ]
        stq = ctx.enter_context(nc.sbuf_tensor("stq", [128, 2 * BPC], f32))
        res = ctx.enter_context(nc.sbuf_tensor("res", [128, BPC], f32))

        pss = [
            ctx.enter_context(nc.psum_tensor(f"ps{b}", [1, 2 * FEAT], f32))
            for b in range(BPC)
        ]
        pts = ctx.enter_context(nc.psum_tensor("pts", [128, 2 * BPC], f32))
        pso = ctx.enter_context(nc.psum_tensor("pso", [128, BPC], f32))

        dsems = [
            ctx.enter_context(nc.semaphore(f"dsem{i}"))
            for i in range(BPC * NCH)
        ]
        csem = ctx.enter_context(nc.semaphore("csem"))    # const DMAs
        msem = ctx.enter_context(nc.semaphore("msem"))    # ones/one_f memsets
        fsem = ctx.enter_context(nc.semaphore("fsem"))    # DVE folds
        pesem = ctx.enter_context(nc.semaphore("pesem"))  # PE milestones
        asem = ctx.enter_context(nc.semaphore("asem"))    # ACT srow copies
        vsem = ctx.enter_context(nc.semaphore("vsem"))    # DVE stq copies
        osem = ctx.enter_context(nc.semaphore("osem"))    # output DMAs
        sem_nums = sorted(
            s.num
            for s in (*dsems, csem, msem, fsem, pesem, asem, vsem, osem)
        )

        def xt(b, j):
            return xts[b * NCH + j]

        def ft(b, j):
            return fts[b * NCH + j]

        with nc.Block() as block:

            @block.sync
            def _(sync):
                # the bulk X stream for the 92-partition block, in
                # consumption order, on the SP HWDGE ring
                for b in range(BPC):
                    for j in range(NCH):
                        c0 = j * CW
                        c1 = c0 + (CW if j < 8 else MW)
                        sync.dma_start(
                            out=xt(b, j)[0:92, :],
                            in_=XA_d[b][:, c0:c1],
                        ).then_inc(dsems[b * NCH + j], 16)

            @block.scalar
            def _(scalar):
                # consts first on the ACT HWDGE ring
                scalar.dma_start(out=w2_sb[:, 0:MEM], in_=W2_d[0]).then_inc(csem, 16)
                scalar.dma_start(out=w2_sb[:, MEM : 2 * MEM], in_=W2_d[1]).then_inc(
                    csem, 16
                )
                scalar.dma_start(out=bias_sb[:, :], in_=bias_d[:, :]).then_inc(csem, 16)
                # the 28-partition block + the slow-engine (half-density)
                # blocks, in consumption order
                for b in range(BPC):
                    for j in range(NCH):
                        c0 = j * CW
                        c1 = c0 + (CW if j < 8 else MW)
                        scalar.dma_start(
                            out=xt(b, j)[96:124, :],
                            in_=XC_d[b][:, c0:c1],
                        ).then_inc(dsems[b * NCH + j], 16)
                        if j <= 3:
                            scalar.dma_start(
                                out=xt(b, j)[92:96, :],
                                in_=XB_d[b][0][:, c0:c1],
                            ).then_inc(dsems[b * NCH + j], 16)
                            scalar.dma_start(
                                out=xt(b, j)[124:128, :],
                                in_=XB_d[b][1][:, c0:c1],
                            ).then_inc(dsems[b * NCH + j], 16)
                        elif j == 4:
                            scalar.dma_start(
                                out=xt(b, j)[92:96, 0:MW],
                                in_=XB_d[b][0][:, 4 * CW : 4 * CW + MW],
                            ).then_inc(dsems[b * NCH + j], 16)
                            scalar.dma_start(
                                out=xt(b, j)[124:128, 0:MW],
                                in_=XB_d[b][1][:, 4 * CW : 4 * CW + MW],
                            ).then_inc(dsems[b * NCH + j], 16)
                            # zero-fill the row range the slow partitions
                            # don't cover (compute memset can't start at
                            # partition 92, so DMA zeros instead)
                            scalar.dma_start(
                                out=xt(b, j)[92:96, MW:CW],
                                in_=zero_d[:, MW:CW],
                            ).then_inc(dsems[b * NCH + j], 16)
                            scalar.dma_start(
                                out=xt(b, j)[124:128, MW:CW],
                                in_=zero_d[:, MW:CW],
                            ).then_inc(dsems[b * NCH + j], 16)
                        else:
                            w = CW if j < 8 else MW
                            scalar.dma_start(
                                out=xt(b, j)[92:96, 0:w],
                                in_=zero_d[:, 0:w],
                            ).then_inc(dsems[b * NCH + j], 16)
                            scalar.dma_start(
                                out=xt(b, j)[124:128, 0:w],
                                in_=zero_d[:, 0:w],
                            ).then_inc(dsems[b * NCH + j], 16)
                # per-batch finale: srow copy, relu, output DMA
                scalar.wait_ge(csem, 48)
                for b in range(BPC):
                    scalar.wait_ge(pesem, 3 * b + 1)
                    nc.scalar.activation(
                        out=srows[b][:, :],
                        in_=pss[b][0:1, :],
                        func=AF.Copy,
                        scale=1.0,
                    ).then_inc(asem, 1)
                    scalar.wait_ge(pesem, 3 * b + 3)
                    nc.scalar.activation(
                        out=res[:, b : b + 1],
                        in_=pso[:, b : b + 1],
                        func=AF.Relu,
                        bias=bias_sb[:, 0:1],
                        scale=1.0,
                    )
                scalar.dma_start(out=out_d[:, :], in_=res[:, :]).then_inc(osem, 16)
                scalar.wait_ge(osem, 16)

            @block.tensor
            def _(pe):
                pe.wait_ge(msem, 1)
                pe.wait_ge(csem, 48)
                for b in range(BPC):
                    # column-sum: folded full chunks, then the raw 2-row
                    # mini chunk (512 cols) to close the accumulation group
                    for j in range(8):
                        pe.wait_ge(fsem, b * 8 + j + 1)
                        nc.tensor.matmul(
                            pss[b][:, :],
                            lhsT=ones16[:, 0:1],
                            rhs=ft(b, j)[:, 0:512],
                            start=(j == 0),
                            stop=False,
                        )
                        nc.tensor.matmul(
                            pss[b][:, :],
                            lhsT=ones16[:, 0:1],
                            rhs=ft(b, j)[:, 512:1024],
                            start=False,
                            stop=False,
                        )
                    pe.wait_ge(dsems[b * NCH + 8], 64)
                    nc.tensor.matmul(
                        pss[b][:, :],
                        lhsT=ones16[:, 0:1],
                        rhs=xt(b, 8)[:, 0:MW],
                        start=False,
                        stop=True,
                    ).then_inc(pesem, 1)  # 3b+1
                    # fold even/odd halves + transpose into pts columns
                    pe.wait_ge(asem, b + 1)
                    last = None
                    for h in range(2):
                        nc.tensor.matmul(
                            pts[:, 2 * b + h : 2 * b + h + 1],
                            lhsT=srows[b][0:1, h * 128 : (h + 1) * 128],
                            rhs=one_f[0:1, 0:1],
                            is_transpose=True,
                            start=True,
                            stop=False,
                        )
                        last = nc.tensor.matmul(
                            pts[:, 2 * b + h : 2 * b + h + 1],
                            lhsT=srows[b][0:1, 256 + h * 128 : 256 + (h + 1) * 128],
                            rhs=one_f[0:1, 0:1],
                            is_transpose=True,
                            start=False,
                            stop=True,
                        )
                    last.then_inc(pesem, 1)  # 3b+2
                    # out_col[b] = W2^T @ s_feat[b]
                    pe.wait_ge(vsem, b + 1)
                    nc.tensor.matmul(
                        pso[:, b : b + 1],
                        lhsT=w2_sb[:, 0:MEM],
                        rhs=stq[:, 2 * b : 2 * b + 1],
                        start=True,
                        stop=False,
                    )
                    nc.tensor.matmul(
                        pso[:, b : b + 1],
                        lhsT=w2_sb[:, MEM : 2 * MEM],
                        rhs=stq[:, 2 * b + 1 : 2 * b + 2],
                        start=False,
                        stop=True,
                    ).then_inc(pesem, 1)  # 3b+3

            @block.vector
            def _(vector):
                nc.vector.memset(ones16[:, :], 1.0)
                nc.vector.memset(one_f[:, :], 1.0).then_inc(msem, 1)
                # fold row pairs: ft = xt[:, :half] + xt[:, half:]
                for b in range(BPC):
                    for j in range(8):
                        thr = 96 if j == 4 else 64
                        vector.wait_ge(dsems[b * NCH + j], thr)
                        half = CW // 2
                        nc.vector.scalar_tensor_tensor(
                            out=ft(b, j)[:, :],
                            in0=xt(b, j)[:, 0:half],
                            scalar=0.0,
                            in1=xt(b, j)[:, half : 2 * half],
                            op0=ALU.add,
                            op1=ALU.add,
                        ).then_inc(fsem, 1)
                # psum transpose columns -> SBUF for the final matmul rhs
                for b in range(BPC):
                    vector.wait_ge(pesem, 3 * b + 2)
                    nc.vector.tensor_copy(
                        out=stq[:, 2 * b : 2 * b + 2], in_=pts[:, 2 * b : 2 * b + 2]
                    ).then_inc(vsem, 1)

            @block.gpsimd
            def _(gpsimd):
                gpsimd.wait_ge(osem, 16)

            nc.all_engine_barrier()
            nc.gpsimd.sem_clear(range(sem_nums[0], sem_nums[-1] + 1))

    if not nc.is_finalized():
        nc.finalize()
    return nc


def kernel(X, mem, Wk, bk, Wv, bv, Wo, bo):
    global _built
    _ensure_axon_hooks()
    from concourse.bass_utils import run_bass_kernel_spmd

    if _built is None:
        _built = _build()
    nc = _built

    X16 = np.asarray(X).astype(np.float16)
    W2 = (
        (np.asarray(Wv, dtype=np.float64) / float(U))
        @ np.asarray(Wo, dtype=np.float64)
    ).astype(np.float32)
    c2 = (
        np.asarray(bv, dtype=np.float64) * (N / float(U))
    ) @ np.asarray(Wo, dtype=np.float64) + np.asarray(bo, dtype=np.float64)
    W2s = np.ascontiguousarray(W2.reshape(2, 128, MEM))
    biasc = np.zeros((MEM, 128), dtype=np.float32)
    biasc[:, 0] = c2.astype(np.float32)

    r0, r1, r2, r3 = 92 * RF, 92 * RF + 4 * RS, 92 * RF + 4 * RS + 28 * RF, N

    in_maps = []
    for i in range(NCORES):
        Xc = X16[i * BPC : (i + 1) * BPC]
        XA = np.ascontiguousarray(Xc[:, 0:r0].reshape(BPC, 92, RF * FEAT))
        XB0 = Xc[:, r0:r1].reshape(BPC, 4, RS * FEAT)
        XC = np.ascontiguousarray(Xc[:, r1:r2].reshape(BPC, 28, RF * FEAT))
        XB1 = Xc[:, r2:r3].reshape(BPC, 4, RS * FEAT)
        XB = np.ascontiguousarray(np.stack([XB0, XB1], axis=1))
        in_maps.append(
            {
                "XA": XA,
                "XB": XB,
                "XC": XC,
                "W2": W2s,
                "biasc": biasc,
                "zeros": np.zeros((4, CHR * FEAT), dtype=np.float16),
            }
        )

    r = run_bass_kernel_spmd(nc, in_maps, list(range(NCORES)))
    kernel._last_results = r

    out = np.empty((B, MEM), dtype=np.float32)
    for i in range(NCORES):
        out[i * BPC : (i + 1) * BPC] = r.results[i]["outT"].T
    return out


# revision 14
# speedup vs baseline: 1.6357x; 1.6357x over previous
"""Trainium2 Bass kernel for nn_MemoryBlock (scatter_memory).

Mathematical identity: softmax over the memory-unit axis U produces rows
that sum to exactly 1, so

    out[b] = relu( mean_u( sum_n attn[b,n,u] * V[b,n,:] ) @ Wo + bo )
           = relu( (sum_n X[b,n,:]) @ W2 + c2 )

with W2 = (Wv/U) @ Wo and c2 = (N/U)*bv @ Wo + bo folded on the host --
the whole K/scores/softmax path cancels algebraically, leaving a
memory-bound column-sum of X plus one tiny matmul.

Device-side choices (per core, data-parallel over batch B):
- X is downcast to fp16 on the host (loose tolerance; colsum error is
  ~2e-4), halving HBM traffic to 8.4 MB/core.
- The HWDGE deals a DMA's per-partition descriptors to SDMA engines in
  equal contiguous blocks, using the largest divisor of the partition
  count that is <= 16.  A 120-partition DMA therefore runs on engines
  0-14 and engine 15 -- the documented straggler that measurably lags
  ~20% on this part -- carries nothing.  Layout: partitions 0-119 hold
  68 rows each, partitions 120-127 hold 4 rows each (delivered by an
  8-partition DMA that lands on engines 0-7), total 8192 rows.
- DVE folds row pairs (fp16 add) before the PE ones-matmul column-sum,
  halving TensorE streaming time so DMA stays the bottleneck.
- Finale: ACT copies the colsum row out of PSUM, PE transposes it into
  columns (folding the even/odd row interleave), one W2 matmul, relu
  with the folded bias, single 1KB output DMA issued by ACT itself.
"""

import contextlib

import numpy as np

B, N, FEAT, MEM, U = 16, 8192, 256, 128, 512
NCORES = 8
BPC = B // NCORES

P_MAIN = 120       # partitions carrying the bulk (SDMA engines 0-14)
P_TAIL = 8         # partitions 120-127 (ride engines 0-7)
R_MAIN = 68        # rows per main partition per batch
R_TAIL = 4         # rows per tail partition per batch
NCH = 9            # 8 full chunks (8 rows) + 1 mini chunk (4 rows)
CHR = 8
CW = CHR * FEAT    # 2048 fp16 cols per full chunk
MW = R_TAIL * FEAT  # 1024 cols in the mini chunk

_built = None


def _ensure_axon_hooks():
    try:
        import antenv.axon_hooks  # noqa: F401
        return
    except ImportError:
        pass
    import sys
    import types

    m = types.ModuleType("antenv.axon_hooks")
    holder = [None]
    m.set_axon_ntff_profile_hook = lambda h: holder.__setitem__(0, h)
    m.get_axon_ntff_profile_hook = lambda: holder[0]
    sys.modules["antenv.axon_hooks"] = m
    try:
        import antenv

        antenv.axon_hooks = m
    except ImportError:
        pass


def _build():
    import concourse.bacc as bacc
    import concourse.mybir as mybir

    f32 = mybir.dt.float32
    f16 = mybir.dt.float16
    AF = mybir.ActivationFunctionType
    ALU = mybir.AluOpType
    nc = bacc.Bacc(None, enable_partition_id=False, monotonic_sem_count=0)

    XA_d = nc.dram_tensor(
        "XA", [BPC, P_MAIN, R_MAIN * FEAT], f16, kind="ExternalInput"
    )
    XE_d = nc.dram_tensor(
        "XE", [BPC, P_TAIL, R_TAIL * FEAT], f16, kind="ExternalInput"
    )
    W2_d = nc.dram_tensor("W2", [2, 128, MEM], f32, kind="ExternalInput")
    # bias padded to 512B/partition rows for line-rate descriptors: col0=c2
    bias_d = nc.dram_tensor("biasc", [MEM, 128], f32, kind="ExternalInput")
    out_d = nc.dram_tensor("outT", [MEM, BPC], f32, kind="ExternalOutput")

    ctx = contextlib.ExitStack()
    with ctx:
        xts, fts = [], []
        for b in range(BPC):
            for j in range(NCH):
                w = CW if j < 8 else MW
                xts.append(
                    ctx.enter_context(nc.sbuf_tensor(f"xt{b}_{j}", [128, w], f16))
                )
                fts.append(
                    ctx.enter_context(
                        nc.sbuf_tensor(f"ft{b}_{j}", [128, w // 2], f16)
                    )
                )
        ones16 = ctx.enter_context(nc.sbuf_tensor("ones16", [128, 1], f16))
        one_f = ctx.enter_context(nc.sbuf_tensor("one_f", [1, 1], f32))
        w2_sb = ctx.enter_context(nc.sbuf_tensor("w2_sb", [128, 2 * MEM], f32))
        bias_sb = ctx.enter_context(nc.sbuf_tensor("bias_sb", [128, 128], f32))
        srows = [
            ctx.enter_context(nc.sbuf_tensor(f"srow{b}", [1, 2 * FEAT], f32))
            for b in range(BPC)
        ]
        stq = ctx.enter_context(nc.sbuf_tensor("stq", [128, 2 * BPC], f32))
        res = ctx.enter_context(nc.sbuf_tensor("res", [128, BPC], f32))

        pss = [
            ctx.enter_context(nc.psum_tensor(f"ps{b}", [1, 2 * FEAT], f32))
            for b in range(BPC)
        ]
        pts = ctx.enter_context(nc.psum_tensor("pts", [128, 2 * BPC], f32))
        pso = ctx.enter_context(nc.psum_tensor("pso", [128, BPC], f32))

        dsems = [
            ctx.enter_context(nc.semaphore(f"dsem{i}"))
            for i in range(BPC * NCH)
        ]
        csem = ctx.enter_context(nc.semaphore("csem"))    # const DMAs
        msem = ctx.enter_context(nc.semaphore("msem"))    # ones/one_f memsets
        fsem = ctx.enter_context(nc.semaphore("fsem"))    # DVE folds
        pesem = ctx.enter_context(nc.semaphore("pesem"))  # PE milestones
        asem = ctx.enter_context(nc.semaphore("asem"))    # ACT srow copies
        vsem = ctx.enter_context(nc.semaphore("vsem"))    # DVE stq copies
        osem = ctx.enter_context(nc.semaphore("osem"))    # output DMA
        sem_nums = sorted(
            s.num
            for s in (*dsems, csem, msem, fsem, pesem, asem, vsem, osem)
        )

        def xt(b, j):
            return xts[b * NCH + j]

        def ft(b, j):
            return fts[b * NCH + j]

        with nc.Block() as block:

            @block.sync
            def _(sync):
                # the bulk X stream (120 partitions -> SDMA engines 0-14),
                # in consumption order, on the SP HWDGE ring
                for b in range(BPC):
                    for j in range(NCH):
                        c0 = j * CW
                        c1 = c0 + (CW if j < 8 else MW)
                        sync.dma_start(
                            out=xt(b, j)[0:P_MAIN, :],
                            in_=XA_d[b][:, c0:c1],
                        ).then_inc(dsems[b * NCH + j], 16)

            @block.scalar
            def _(scalar):
                # consts + the tail-partition mini rects on the ACT ring
                scalar.dma_start(out=w2_sb[:, 0:MEM], in_=W2_d[0]).then_inc(csem, 16)
                scalar.dma_start(out=w2_sb[:, MEM : 2 * MEM], in_=W2_d[1]).then_inc(
                    csem, 16
                )
                scalar.dma_start(out=bias_sb[:, :], in_=bias_d[:, :]).then_inc(csem, 16)
                for b in range(BPC):
                    scalar.dma_start(
                        out=xt(b, 8)[P_MAIN:128, :],
                        in_=XE_d[b],
                    ).then_inc(dsems[b * NCH + 8], 16)
                # per-batch finale: srow copy, relu; single output DMA
                scalar.wait_ge(csem, 48)
                for b in range(BPC):
                    scalar.wait_ge(pesem, 3 * b + 1)
                    nc.scalar.activation(
                        out=srows[b][:, :],
                        in_=pss[b][0:1, :],
                        func=AF.Copy,
                        scale=1.0,
                    ).then_inc(asem, 1)
                    scalar.wait_ge(pesem, 3 * b + 3)
                    nc.scalar.activation(
                        out=res[:, b : b + 1],
                        in_=pso[:, b : b + 1],
                        func=AF.Relu,
                        bias=bias_sb[:, 0:1],
                        scale=1.0,
                    )
                scalar.dma_start(out=out_d[:, :], in_=res[:, :]).then_inc(osem, 16)
                scalar.wait_ge(osem, 16)

            @block.tensor
            def _(pe):
                pe.wait_ge(msem, 1)
                pe.wait_ge(csem, 48)
                for b in range(BPC):
                    # column-sum of the folded chunks (contraction over the
                    # 120 main partitions; the mini chunk uses all 128)
                    for j in range(8):
                        pe.wait_ge(fsem, b * NCH + j + 1)
                        nc.tensor.matmul(
                            pss[b][:, :],
                            lhsT=ones16[0:P_MAIN, 0:1],
                            rhs=ft(b, j)[0:P_MAIN, 0:512],
                            start=(j == 0),
                            stop=False,
                        )
                        nc.tensor.matmul(
                            pss[b][:, :],
                            lhsT=ones16[0:P_MAIN, 0:1],
                            rhs=ft(b, j)[0:P_MAIN, 512:1024],
                            start=False,
                            stop=False,
                        )
                    pe.wait_ge(fsem, b * NCH + 9)
                    nc.tensor.matmul(
                        pss[b][:, :],
                        lhsT=ones16[:, 0:1],
                        rhs=ft(b, 8)[:, 0:512],
                        start=False,
                        stop=True,
                    ).then_inc(pesem, 1)  # 3b+1
                    # fold even/odd halves + transpose into pts columns
                    pe.wait_ge(asem, b + 1)
                    last = None
                    for h in range(2):
                        nc.tensor.matmul(
                            pts[:, 2 * b + h : 2 * b + h + 1],
                            lhsT=srows[b][0:1, h * 128 : (h + 1) * 128],
                            rhs=one_f[0:1, 0:1],
                            is_transpose=True,
                            start=True,
                            stop=False,
                        )
                        last = nc.tensor.matmul(
                            pts[:, 2 * b + h : 2 * b + h + 1],
                            lhsT=srows[b][0:1, 256 + h * 128 : 256 + (h + 1) * 128],
                            rhs=one_f[0:1, 0:1],
                            is_transpose=True,
                            start=False,
                            stop=True,
                        )
                    last.then_inc(pesem, 1)  # 3b+2
                    # out_col[b] = W2^T @ s_feat[b]
                    pe.wait_ge(vsem, b + 1)
                    nc.tensor.matmul(
                        pso[:, b : b + 1],
                        lhsT=w2_sb[:, 0:MEM],
                        rhs=stq[:, 2 * b : 2 * b + 1],
                        start=True,
                        stop=False,
                    )
                    nc.tensor.matmul(
                        pso[:, b : b + 1],
                        lhsT=w2_sb[:, MEM : 2 * MEM],
                        rhs=stq[:, 2 * b + 1 : 2 * b + 2],
                        start=False,
                        stop=True,
                    ).then_inc(pesem, 1)  # 3b+3

            @block.vector
            def _(vector):
                nc.vector.memset(ones16[:, :], 1.0)
                nc.vector.memset(one_f[:, :], 1.0).then_inc(msem, 1)
                # fold row pairs: ft = xt[:, :half] + xt[:, half:]
                for b in range(BPC):
                    for j in range(NCH):
                        vector.wait_ge(dsems[b * NCH + j], 32 if j == 8 else 16)
                        if j < 8:
                            nc.vector.scalar_tensor_tensor(
                                out=ft(b, j)[0:P_MAIN, :],
                                in0=xt(b, j)[0:P_MAIN, 0:1024],
                                scalar=0.0,
                                in1=xt(b, j)[0:P_MAIN, 1024:2048],
                                op0=ALU.add,
                                op1=ALU.add,
                            ).then_inc(fsem, 1)
                        else:
                            nc.vector.scalar_tensor_tensor(
                                out=ft(b, j)[:, :],
                                in0=xt(b, j)[:, 0:512],
                                scalar=0.0,
                                in1=xt(b, j)[:, 512:1024],
                                op0=ALU.add,
                                op1=ALU.add,
                            ).then_inc(fsem, 1)
                # psum transpose columns -> SBUF for the final matmul rhs
                for b in range(BPC):
                    vector.wait_ge(pesem, 3 * b + 2)
                    nc.vector.tensor_copy(
                        out=stq[:, 2 * b : 2 * b + 2], in_=pts[:, 2 * b : 2 * b + 2]
                    ).then_inc(vsem, 1)

            @block.gpsimd
            def _(gpsimd):
                gpsimd.wait_ge(osem, 16)

            nc.all_engine_barrier()
            nc.gpsimd.sem_clear(range(sem_nums[0], sem_nums[-1] + 1))

    if not nc.is_finalized():
        nc.finalize()
    return nc


def kernel(X, mem, Wk, bk, Wv, bv, Wo, bo):
    global _built
    _ensure_axon_hooks()
    from concourse.bass_utils import run_bass_kernel_spmd

    if _built is None:
        _built = _build()
    nc = _built

    X16 = np.asarray(X).astype(np.float16)
    W2 = (
        (np.asarray(Wv, dtype=np.float64) / float(U))
        @ np.asarray(Wo, dtype=np.float64)
    ).astype(np.float32)
    c2 = (
        np.asarray(bv, dtype=np.float64) * (N / float(U))
    ) @ np.asarray(Wo, dtype=np.float64) + np.asarray(bo, dtype=np.float64)
    W2s = np.ascontiguousarray(W2.reshape(2, 128, MEM))
    biasc = np.zeros((MEM, 128), dtype=np.float32)
    biasc[:, 0] = c2.astype(np.float32)

    r0 = P_MAIN * R_MAIN  # 8160

    in_maps = []
    for i in range(NCORES):
        Xc = X16[i * BPC : (i + 1) * BPC]
        XA = np.ascontiguousarray(Xc[:, 0:r0].reshape(BPC, P_MAIN, R_MAIN * FEAT))
        XE = np.ascontiguousarray(
            Xc[:, r0:N].reshape(BPC, P_TAIL, R_TAIL * FEAT)
        )
        in_maps.append({"XA": XA, "XE": XE, "W2": W2s, "biasc": biasc})

    r = run_bass_kernel_spmd(nc, in_maps, list(range(NCORES)))
    kernel._last_results = r

    out = np.empty((B, MEM), dtype=np.float32)
    for i in range(NCORES):
        out[i * BPC : (i + 1) * BPC] = r.results[i]["outT"].T
    return out


# revision 16
# speedup vs baseline: 1.6369x; 1.0007x over previous
"""Trainium2 Bass kernel for nn_MemoryBlock (scatter_memory).

Mathematical identity: softmax over the memory-unit axis U produces rows
that sum to exactly 1, so

    out[b] = relu( mean_u( sum_n attn[b,n,u] * V[b,n,:] ) @ Wo + bo )
           = relu( (sum_n X[b,n,:]) @ W2 + c2 )

with W2 = (Wv/U) @ Wo and c2 = (N/U)*bv @ Wo + bo folded on the host --
the whole K/scores/softmax path cancels algebraically, leaving a
memory-bound column-sum of X plus one tiny matmul.

Device-side choices (per core, data-parallel over batch B):
- X is downcast to fp16 on the host (loose tolerance; colsum error is
  ~2e-4), halving HBM traffic to 8.4 MB/core.  The fp16 ones-matmul
  also streams 2 cols/cycle so TensorE keeps up with the DMA stream
  without any pre-reduction.
- The HWDGE deals a DMA's per-partition descriptors to SDMA engines in
  equal contiguous blocks, using the largest divisor of the partition
  count that is <= 16 (measured).  A 120-partition DMA therefore runs
  on engines 0-14, and engine 15 -- the documented straggler, measured
  ~20% slow here -- carries nothing.  Partitions 0-119 hold 68 rows
  per batch; partitions 120-127 hold 4 rows, delivered by one tiny
  8-partition DMA (engines 0-7) per batch.
- 16-row chunks = 8 KB per-partition descriptors (4 KB descriptors
  measured ~40% slower per engine).
- Finale per batch: ACT copies the colsum row from PSUM, PE transposes
  it into columns folding the even/odd interleave, one W2 matmul, relu
  with the folded bias, and ACT issues that batch's 512B output DMA
  immediately -- batch 0's output completes mid-stream, only batch 1's
  chain is in the tail.
"""

import contextlib

import numpy as np

B, N, FEAT, MEM, U = 16, 8192, 256, 128, 512
NCORES = 8
BPC = B // NCORES

P_MAIN = 120       # partitions carrying the bulk (SDMA engines 0-14)
P_TAIL = 8         # partitions 120-127 (ride engines 0-7)
R_MAIN = 68        # rows per main partition per batch
R_TAIL = 4         # rows per tail partition per batch
NCH = 5            # 4 full chunks (16 rows) + 1 mini chunk (4 rows)
CHR = 16
CW = CHR * FEAT    # 4096 fp16 cols per full chunk (8 KB)
MW = R_TAIL * FEAT  # 1024 cols in the mini chunk

_built = None


def _ensure_axon_hooks():
    try:
        import antenv.axon_hooks  # noqa: F401
        return
    except ImportError:
        pass
    import sys
    import types

    m = types.ModuleType("antenv.axon_hooks")
    holder = [None]
    m.set_axon_ntff_profile_hook = lambda h: holder.__setitem__(0, h)
    m.get_axon_ntff_profile_hook = lambda: holder[0]
    sys.modules["antenv.axon_hooks"] = m
    try:
        import antenv

        antenv.axon_hooks = m
    except ImportError:
        pass


def _build():
    import concourse.bacc as bacc
    import concourse.mybir as mybir

    f32 = mybir.dt.float32
    f16 = mybir.dt.float16
    AF = mybir.ActivationFunctionType
    nc = bacc.Bacc(None, enable_partition_id=False, monotonic_sem_count=0)

    XA_d = nc.dram_tensor(
        "XA", [BPC, P_MAIN, R_MAIN * FEAT], f16, kind="ExternalInput"
    )
    XE_d = nc.dram_tensor(
        "XE", [BPC, P_TAIL, R_TAIL * FEAT], f16, kind="ExternalInput"
    )
    W2_d = nc.dram_tensor("W2", [2, 128, MEM], f32, kind="ExternalInput")
    # bias padded to 512B/partition rows for line-rate descriptors: col0=c2
    bias_d = nc.dram_tensor("biasc", [MEM, 128], f32, kind="ExternalInput")
    outs_d = [
        nc.dram_tensor(f"out{b}", [MEM, 1], f32, kind="ExternalOutput")
        for b in range(BPC)
    ]

    ctx = contextlib.ExitStack()
    with ctx:
        xts = []
        for b in range(BPC):
            for j in range(NCH):
                w = CW if j < NCH - 1 else MW
                xts.append(
                    ctx.enter_context(nc.sbuf_tensor(f"xt{b}_{j}", [128, w], f16))
                )
        ones16 = ctx.enter_context(nc.sbuf_tensor("ones16", [128, 1], f16))
        one_f = ctx.enter_context(nc.sbuf_tensor("one_f", [1, 1], f32))
        w2_sb = ctx.enter_context(nc.sbuf_tensor("w2_sb", [128, 2 * MEM], f32))
        bias_sb = ctx.enter_context(nc.sbuf_tensor("bias_sb", [128, 128], f32))
        srows = [
            ctx.enter_context(nc.sbuf_tensor(f"srow{b}", [1, 2 * FEAT], f32))
            for b in range(BPC)
        ]
        stq = ctx.enter_context(nc.sbuf_tensor("stq", [128, 2 * BPC], f32))
        res = ctx.enter_context(nc.sbuf_tensor("res", [128, BPC], f32))

        pss = [
            ctx.enter_context(nc.psum_tensor(f"ps{b}", [1, 2 * FEAT], f32))
            for b in range(BPC)
        ]
        pts = ctx.enter_context(nc.psum_tensor("pts", [128, 2 * BPC], f32))
        pso = ctx.enter_context(nc.psum_tensor("pso", [128, BPC], f32))

        dsems = [
            ctx.enter_context(nc.semaphore(f"dsem{i}"))
            for i in range(BPC * NCH)
        ]
        csem = ctx.enter_context(nc.semaphore("csem"))    # const DMAs
        msem = ctx.enter_context(nc.semaphore("msem"))    # ones/one_f memsets
        pesem = ctx.enter_context(nc.semaphore("pesem"))  # PE milestones
        asem = ctx.enter_context(nc.semaphore("asem"))    # ACT srow copies
        vsem = ctx.enter_context(nc.semaphore("vsem"))    # DVE stq copies
        osem = ctx.enter_context(nc.semaphore("osem"))    # output DMAs
        sem_nums = sorted(
            s.num for s in (*dsems, csem, msem, pesem, asem, vsem, osem)
        )

        def xt(b, j):
            return xts[b * NCH + j]

        with nc.Block() as block:

            @block.sync
            def _(sync):
                # the bulk X stream (120 partitions -> SDMA engines 0-14),
                # in consumption order, on the SP HWDGE ring
                for b in range(BPC):
                    for j in range(NCH):
                        c0 = j * CW
                        c1 = c0 + (CW if j < NCH - 1 else MW)
                        sync.dma_start(
                            out=xt(b, j)[0:P_MAIN, :],
                            in_=XA_d[b][:, c0:c1],
                        ).then_inc(dsems[b * NCH + j], 16)

            @block.scalar
            def _(scalar):
                # consts + the tail-partition mini rects on the ACT ring
                scalar.dma_start(out=w2_sb[:, 0:MEM], in_=W2_d[0]).then_inc(csem, 16)
                scalar.dma_start(out=w2_sb[:, MEM : 2 * MEM], in_=W2_d[1]).then_inc(
                    csem, 16
                )
                scalar.dma_start(out=bias_sb[:, :], in_=bias_d[:, :]).then_inc(csem, 16)
                for b in range(BPC):
                    scalar.dma_start(
                        out=xt(b, NCH - 1)[P_MAIN:128, :],
                        in_=XE_d[b],
                    ).then_inc(dsems[b * NCH + NCH - 1], 16)
                # per-batch finale: srow copy, relu, 512B output DMA
                scalar.wait_ge(csem, 48)
                for b in range(BPC):
                    scalar.wait_ge(pesem, 3 * b + 1)
                    nc.scalar.activation(
                        out=srows[b][:, :],
                        in_=pss[b][0:1, :],
                        func=AF.Copy,
                        scale=1.0,
                    ).then_inc(asem, 1)
                    scalar.wait_ge(pesem, 3 * b + 3)
                    nc.scalar.activation(
                        out=res[:, b : b + 1],
                        in_=pso[:, b : b + 1],
                        func=AF.Relu,
                        bias=bias_sb[:, 0:1],
                        scale=1.0,
                    )
                    scalar.dma_start(
                        out=outs_d[b][:, :], in_=res[:, b : b + 1]
                    ).then_inc(osem, 16)
                scalar.wait_ge(osem, 16 * BPC)

            @block.tensor
            def _(pe):
                pe.wait_ge(msem, 1)
                pe.wait_ge(csem, 48)
                for b in range(BPC):
                    # column-sum straight off the fp16 chunks (contraction
                    # over the 120 main partitions; mini uses all 128)
                    k = 0
                    nmm = (NCH - 1) * (CW // 512) + MW // 512
                    for j in range(NCH - 1):
                        pe.wait_ge(dsems[b * NCH + j], 16)
                        for m in range(CW // 512):
                            nc.tensor.matmul(
                                pss[b][:, :],
                                lhsT=ones16[0:P_MAIN, 0:1],
                                rhs=xt(b, j)[0:P_MAIN, m * 512 : (m + 1) * 512],
                                start=(k == 0),
                                stop=False,
                            )
                            k += 1
                    pe.wait_ge(dsems[b * NCH + NCH - 1], 32)
                    lastc = None
                    for m in range(MW // 512):
                        k += 1
                        lastc = nc.tensor.matmul(
                            pss[b][:, :],
                            lhsT=ones16[:, 0:1],
                            rhs=xt(b, NCH - 1)[:, m * 512 : (m + 1) * 512],
                            start=False,
                            stop=(k == nmm),
                        )
                    lastc.then_inc(pesem, 1)  # 3b+1
                    # fold even/odd halves + transpose into pts columns
                    pe.wait_ge(asem, b + 1)
                    last = None
                    for h in range(2):
                        nc.tensor.matmul(
                            pts[:, 2 * b + h : 2 * b + h + 1],
                            lhsT=srows[b][0:1, h * 128 : (h + 1) * 128],
                            rhs=one_f[0:1, 0:1],
                            is_transpose=True,
                            start=True,
                            stop=False,
                        )
                        last = nc.tensor.matmul(
                            pts[:, 2 * b + h : 2 * b + h + 1],
                            lhsT=srows[b][0:1, 256 + h * 128 : 256 + (h + 1) * 128],
                            rhs=one_f[0:1, 0:1],
                            is_transpose=True,
                            start=False,
                            stop=True,
                        )
                    last.then_inc(pesem, 1)  # 3b+2
                    # out_col[b] = W2^T @ s_feat[b]
                    pe.wait_ge(vsem, b + 1)
                    nc.tensor.matmul(
                        pso[:, b : b + 1],
                        lhsT=w2_sb[:, 0:MEM],
                        rhs=stq[:, 2 * b : 2 * b + 1],
                        start=True,
                        stop=False,
                    )
                    nc.tensor.matmul(
                        pso[:, b : b + 1],
                        lhsT=w2_sb[:, MEM : 2 * MEM],
                        rhs=stq[:, 2 * b + 1 : 2 * b + 2],
                        start=False,
                        stop=True,
                    ).then_inc(pesem, 1)  # 3b+3

            @block.vector
            def _(vector):
                nc.vector.memset(ones16[:, :], 1.0)
                nc.vector.memset(one_f[:, :], 1.0).then_inc(msem, 1)
                # psum transpose columns -> SBUF for the final matmul rhs
                for b in range(BPC):
                    vector.wait_ge(pesem, 3 * b + 2)
                    nc.vector.tensor_copy(
                        out=stq[:, 2 * b : 2 * b + 2], in_=pts[:, 2 * b : 2 * b + 2]
                    ).then_inc(vsem, 1)

            @block.gpsimd
            def _(gpsimd):
                gpsimd.wait_ge(osem, 16 * BPC)

            nc.all_engine_barrier()
            nc.gpsimd.sem_clear(range(sem_nums[0], sem_nums[-1] + 1))

    if not nc.is_finalized():
        nc.finalize()
    return nc


def kernel(X, mem, Wk, bk, Wv, bv, Wo, bo):
    global _built
    _ensure_axon_hooks()
    from concourse.bass_utils import run_bass_kernel_spmd

    if _built is None:
        _built = _build()
    nc = _built

    X16 = np.asarray(X).astype(np.float16)
    W2 = (
        (np.asarray(Wv, dtype=np.float64) / float(U))
        @ np.asarray(Wo, dtype=np.float64)
    ).astype(np.float32)
    c2 = (
        np.asarray(bv, dtype=np.float64) * (N / float(U))
    ) @ np.asarray(Wo, dtype=np.float64) + np.asarray(bo, dtype=np.float64)
    W2s = np.ascontiguousarray(W2.reshape(2, 128, MEM))
    biasc = np.zeros((MEM, 128), dtype=np.float32)
    biasc[:, 0] = c2.astype(np.float32)

    r0 = P_MAIN * R_MAIN  # 8160

    in_maps = []
    for i in range(NCORES):
        Xc = X16[i * BPC : (i + 1) * BPC]
        XA = np.ascontiguousarray(Xc[:, 0:r0].reshape(BPC, P_MAIN, R_MAIN * FEAT))
        XE = np.ascontiguousarray(
            Xc[:, r0:N].reshape(BPC, P_TAIL, R_TAIL * FEAT)
        )
        in_maps.append({"XA": XA, "XE": XE, "W2": W2s, "biasc": biasc})

    r = run_bass_kernel_spmd(nc, in_maps, list(range(NCORES)))
    kernel._last_results = r

    out = np.empty((B, MEM), dtype=np.float32)
    for i in range(NCORES):
        for b in range(BPC):
            out[i * BPC + b] = r.results[i][f"out{b}"][:, 0]
    return out


# revision 17
# speedup vs baseline: 2.0054x; 1.2251x over previous
"""Trainium2 Bass kernel for nn_MemoryBlock (scatter_memory).

Mathematical identity: softmax over the memory-unit axis U produces rows
that sum to exactly 1, so

    out[b] = relu( mean_u( sum_n attn[b,n,u] * V[b,n,:] ) @ Wo + bo )
           = relu( (sum_n X[b,n,:]) @ W2 + c2 )

with W2 = (Wv/U) @ Wo and c2 = (N/U)*bv @ Wo + bo folded on the host --
the whole K/scores/softmax path cancels algebraically, leaving a
memory-bound column-sum of X plus one tiny matmul.

Device-side choices (per core, data-parallel over batch B):
- X is downcast to fp16 on the host (loose tolerance; colsum error is
  ~2e-4), halving HBM traffic to 8.4 MB/core.  The fp16 ones-matmul
  streams 2 cols/cycle, so TensorE consumes the raw chunks directly
  (no pre-reduction needed to keep up with the DMA stream).
- DMAs are strictly 128-partition: measured, non-128 partition counts
  fall off the DGE's optimized engine swizzle and halve the per-engine
  SDMA rate.  Layout is the classic [128 partitions x 64 rows]/batch.
- 16-row chunks = 8 KB per-partition descriptors (4 KB descriptors
  measured ~40% slower per engine).
- Finale per batch: ACT copies the colsum row from PSUM, PE transposes
  it into columns folding the even/odd interleave, one W2 matmul, relu
  with the folded bias, and ACT issues that batch's 512B output DMA
  immediately -- batch 0's output completes mid-stream, only batch 1's
  chain is in the tail.
"""

import contextlib

import numpy as np

B, N, FEAT, MEM, U = 16, 8192, 256, 128, 512
NCORES = 8
BPC = B // NCORES

RPP = N // 128     # 64 rows per partition per batch
NCH = 4            # chunks per batch
CHR = RPP // NCH   # 16 rows per chunk
CW = CHR * FEAT    # 4096 fp16 cols per chunk (8 KB per partition)

_built = None


def _ensure_axon_hooks():
    try:
        import antenv.axon_hooks  # noqa: F401
        return
    except ImportError:
        pass
    import sys
    import types

    m = types.ModuleType("antenv.axon_hooks")
    holder = [None]
    m.set_axon_ntff_profile_hook = lambda h: holder.__setitem__(0, h)
    m.get_axon_ntff_profile_hook = lambda: holder[0]
    sys.modules["antenv.axon_hooks"] = m
    try:
        import antenv

        antenv.axon_hooks = m
    except ImportError:
        pass


def _build():
    import concourse.bacc as bacc
    import concourse.mybir as mybir

    f32 = mybir.dt.float32
    f16 = mybir.dt.float16
    AF = mybir.ActivationFunctionType
    nc = bacc.Bacc(None, enable_partition_id=False, monotonic_sem_count=0)

    X_d = nc.dram_tensor("Xs", [BPC, N, FEAT], f16, kind="ExternalInput")
    W2_d = nc.dram_tensor("W2", [2, 128, MEM], f32, kind="ExternalInput")
    # bias padded to 512B/partition rows for line-rate descriptors: col0=c2
    bias_d = nc.dram_tensor("biasc", [MEM, 128], f32, kind="ExternalInput")
    outs_d = [
        nc.dram_tensor(f"out{b}", [MEM, 1], f32, kind="ExternalOutput")
        for b in range(BPC)
    ]

    ctx = contextlib.ExitStack()
    with ctx:
        xts = [
            ctx.enter_context(nc.sbuf_tensor(f"xt{i}", [128, CW], f16))
            for i in range(BPC * NCH)
        ]
        ones16 = ctx.enter_context(nc.sbuf_tensor("ones16", [128, 1], f16))
        one_f = ctx.enter_context(nc.sbuf_tensor("one_f", [1, 1], f32))
        w2_sb = ctx.enter_context(nc.sbuf_tensor("w2_sb", [128, 2 * MEM], f32))
        bias_sb = ctx.enter_context(nc.sbuf_tensor("bias_sb", [128, 128], f32))
        srows = [
            ctx.enter_context(nc.sbuf_tensor(f"srow{b}", [1, 2 * FEAT], f32))
            for b in range(BPC)
        ]
        stq = ctx.enter_context(nc.sbuf_tensor("stq", [128, 2 * BPC], f32))
        res = ctx.enter_context(nc.sbuf_tensor("res", [128, BPC], f32))

        pss = [
            ctx.enter_context(nc.psum_tensor(f"ps{b}", [1, 2 * FEAT], f32))
            for b in range(BPC)
        ]
        pts = ctx.enter_context(nc.psum_tensor("pts", [128, 2 * BPC], f32))
        pso = ctx.enter_context(nc.psum_tensor("pso", [128, BPC], f32))

        dsems = [
            ctx.enter_context(nc.semaphore(f"dsem{i}"))
            for i in range(BPC * NCH)
        ]
        csem = ctx.enter_context(nc.semaphore("csem"))    # const DMAs
        msem = ctx.enter_context(nc.semaphore("msem"))    # ones/one_f memsets
        pesem = ctx.enter_context(nc.semaphore("pesem"))  # PE milestones
        asem = ctx.enter_context(nc.semaphore("asem"))    # ACT srow copies
        vsem = ctx.enter_context(nc.semaphore("vsem"))    # DVE stq copies
        osem = ctx.enter_context(nc.semaphore("osem"))    # output DMAs
        sem_nums = sorted(
            s.num for s in (*dsems, csem, msem, pesem, asem, vsem, osem)
        )

        def xt(b, j):
            return xts[b * NCH + j]

        with nc.Block() as block:

            @block.sync
            def _(sync):
                # X chunk stream in consumption order on the SP HWDGE ring
                for b in range(BPC):
                    Xb = X_d[b].rearrange("(p r) f -> p (r f)", p=128)
                    for j in range(NCH):
                        sync.dma_start(
                            out=xt(b, j)[:, :],
                            in_=Xb[:, j * CW : (j + 1) * CW],
                        ).then_inc(dsems[b * NCH + j], 16)

            @block.scalar
            def _(scalar):
                # consts on the ACT ring
                scalar.dma_start(out=w2_sb[:, 0:MEM], in_=W2_d[0]).then_inc(csem, 16)
                scalar.dma_start(out=w2_sb[:, MEM : 2 * MEM], in_=W2_d[1]).then_inc(
                    csem, 16
                )
                scalar.dma_start(out=bias_sb[:, :], in_=bias_d[:, :]).then_inc(csem, 16)
                # per-batch finale: srow copy, relu, 512B output DMA
                scalar.wait_ge(csem, 48)
                for b in range(BPC):
                    scalar.wait_ge(pesem, 3 * b + 1)
                    nc.scalar.activation(
                        out=srows[b][:, :],
                        in_=pss[b][0:1, :],
                        func=AF.Copy,
                        scale=1.0,
                    ).then_inc(asem, 1)
                    scalar.wait_ge(pesem, 3 * b + 3)
                    nc.scalar.activation(
                        out=res[:, b : b + 1],
                        in_=pso[:, b : b + 1],
                        func=AF.Relu,
                        bias=bias_sb[:, 0:1],
                        scale=1.0,
                    )
                    scalar.dma_start(
                        out=outs_d[b][:, :], in_=res[:, b : b + 1]
                    ).then_inc(osem, 16)
                scalar.wait_ge(osem, 16 * BPC)

            @block.tensor
            def _(pe):
                pe.wait_ge(msem, 1)
                pe.wait_ge(csem, 48)
                nmm = NCH * (CW // 512)
                for b in range(BPC):
                    # column-sum straight off the fp16 chunks
                    k = 0
                    lastc = None
                    for j in range(NCH):
                        pe.wait_ge(dsems[b * NCH + j], 16)
                        for m in range(CW // 512):
                            k += 1
                            lastc = nc.tensor.matmul(
                                pss[b][:, :],
                                lhsT=ones16[:, 0:1],
                                rhs=xt(b, j)[:, (m * 512) : (m + 1) * 512],
                                start=(k == 1),
                                stop=(k == nmm),
                            )
                    lastc.then_inc(pesem, 1)  # 3b+1
                    # fold even/odd halves + transpose into pts columns
                    pe.wait_ge(asem, b + 1)
                    last = None
                    for h in range(2):
                        nc.tensor.matmul(
                            pts[:, 2 * b + h : 2 * b + h + 1],
                            lhsT=srows[b][0:1, h * 128 : (h + 1) * 128],
                            rhs=one_f[0:1, 0:1],
                            is_transpose=True,
                            start=True,
                            stop=False,
                        )
                        last = nc.tensor.matmul(
                            pts[:, 2 * b + h : 2 * b + h + 1],
                            lhsT=srows[b][0:1, 256 + h * 128 : 256 + (h + 1) * 128],
                            rhs=one_f[0:1, 0:1],
                            is_transpose=True,
                            start=False,
                            stop=True,
                        )
                    last.then_inc(pesem, 1)  # 3b+2
                    # out_col[b] = W2^T @ s_feat[b]
                    pe.wait_ge(vsem, b + 1)
                    nc.tensor.matmul(
                        pso[:, b : b + 1],
                        lhsT=w2_sb[:, 0:MEM],
                        rhs=stq[:, 2 * b : 2 * b + 1],
                        start=True,
                        stop=False,
                    )
                    nc.tensor.matmul(
                        pso[:, b : b + 1],
                        lhsT=w2_sb[:, MEM : 2 * MEM],
                        rhs=stq[:, 2 * b + 1 : 2 * b + 2],
                        start=False,
                        stop=True,
                    ).then_inc(pesem, 1)  # 3b+3

            @block.vector
            def _(vector):
                nc.vector.memset(ones16[:, :], 1.0)
                nc.vector.memset(one_f[:, :], 1.0).then_inc(msem, 1)
                # psum transpose columns -> SBUF for the final matmul rhs
                for b in range(BPC):
                    vector.wait_ge(pesem, 3 * b + 2)
                    nc.vector.tensor_copy(
                        out=stq[:, 2 * b : 2 * b + 2], in_=pts[:, 2 * b : 2 * b + 2]
                    ).then_inc(vsem, 1)

            @block.gpsimd
            def _(gpsimd):
                gpsimd.wait_ge(osem, 16 * BPC)

            nc.all_engine_barrier()
            nc.gpsimd.sem_clear(range(sem_nums[0], sem_nums[-1] + 1))

    if not nc.is_finalized():
        nc.finalize()
    return nc


def kernel(X, mem, Wk, bk, Wv, bv, Wo, bo):
    global _built
    _ensure_axon_hooks()
    from concourse.bass_utils import run_bass_kernel_spmd

    if _built is None:
        _built = _build()
    nc = _built

    X16 = np.asarray(X).astype(np.float16)
    W2 = (
        (np.asarray(Wv, dtype=np.float64) / float(U))
        @ np.asarray(Wo, dtype=np.float64)
    ).astype(np.float32)
    c2 = (
        np.asarray(bv, dtype=np.float64) * (N / float(U))
    ) @ np.asarray(Wo, dtype=np.float64) + np.asarray(bo, dtype=np.float64)
    W2s = np.ascontiguousarray(W2.reshape(2, 128, MEM))
    biasc = np.zeros((MEM, 128), dtype=np.float32)
    biasc[:, 0] = c2.astype(np.float32)

    in_maps = [
        {
            "Xs": np.ascontiguousarray(X16[i * BPC : (i + 1) * BPC]),
            "W2": W2s,
            "biasc": biasc,
        }
        for i in range(NCORES)
    ]
    r = run_bass_kernel_spmd(nc, in_maps, list(range(NCORES)))
    kernel._last_results = r

    out = np.empty((B, MEM), dtype=np.float32)
    for i in range(NCORES):
        for b in range(BPC):
            out[i * BPC + b] = r.results[i][f"out{b}"][:, 0]
    return out


# revision 25
# speedup vs baseline: 2.5925x; 1.2928x over previous
"""Trainium2 Bass kernel for nn_MemoryBlock (scatter_memory).

Mathematical identity: softmax over the memory-unit axis U produces rows
that sum to exactly 1, so

    out[b] = relu( mean_u( sum_n attn[b,n,u] * V[b,n,:] ) @ Wo + bo )
           = relu( (sum_n X[b,n,:]) @ W2 + c2 )

with W2 = (Wv/U) @ Wo and c2 = (N/U)*bv @ Wo + bo folded on the host --
the whole K/scores/softmax path cancels algebraically, leaving a
memory-bound column-sum of X plus one tiny matmul.

Device-side choices (per core, data-parallel over batch B):
- X is downcast to fp16 on the host (loose tolerance; colsum error is
  ~2e-4), halving HBM traffic to 8.4 MB/core.  The fp16 ones-matmul
  streams 2 cols/cycle, so TensorE consumes the raw chunks directly
  (no pre-reduction needed to keep up with the DMA stream).
- DMAs are strictly 128-partition: measured, non-128 partition counts
  fall off the DGE's optimized engine swizzle and halve the per-engine
  SDMA rate.  Layout is the classic [128 partitions x 64 rows]/batch.
- 16-row chunks = 8 KB per-partition descriptors (4 KB descriptors
  measured ~40% slower per engine).
- Finale per batch: ACT copies the colsum row from PSUM, PE transposes
  it into columns folding the even/odd interleave, one W2 matmul, relu
  with the folded bias, and ACT issues that batch's 512B output DMA
  immediately -- batch 0's output completes mid-stream, only batch 1's
  chain is in the tail.
"""

import contextlib

import numpy as np

B, N, FEAT, MEM, U = 16, 8192, 256, 128, 512
NCORES = 8
BPC = B // NCORES

RPP = N // 128     # 64 rows per partition per batch
NCH = 4            # chunks per batch
CHR = RPP // NCH   # 16 rows per chunk
CW = CHR * FEAT    # 4096 fp16 cols per chunk (8 KB per partition)

_built = None


def _ensure_axon_hooks():
    try:
        import antenv.axon_hooks  # noqa: F401
        return
    except ImportError:
        pass
    import sys
    import types

    m = types.ModuleType("antenv.axon_hooks")
    holder = [None]
    m.set_axon_ntff_profile_hook = lambda h: holder.__setitem__(0, h)
    m.get_axon_ntff_profile_hook = lambda: holder[0]
    sys.modules["antenv.axon_hooks"] = m
    try:
        import antenv

        antenv.axon_hooks = m
    except ImportError:
        pass


def _build():
    import concourse.bacc as bacc
    import concourse.mybir as mybir

    f32 = mybir.dt.float32
    f16 = mybir.dt.float16
    AF = mybir.ActivationFunctionType
    nc = bacc.Bacc(None, enable_partition_id=False, monotonic_sem_count=0)

    X_d = nc.dram_tensor("Xs", [BPC, N, FEAT], f16, kind="ExternalInput")
    # all consts in one line-rate DMA: cols 0:128 = W2 half0, 128:256 =
    # W2 half1, col 256 = c2, padded to 384 cols (1.5KB/partition)
    cst_d = nc.dram_tensor("consts", [128, 384], f32, kind="ExternalInput")
    # outputs padded to 512B/partition: sub-512B HBM writes RMW and the
    # write receipt (which gates the DMA's semaphore) takes 3-7us
    outs_d = [
        nc.dram_tensor(f"out{b}", [MEM, 128], f32, kind="ExternalOutput")
        for b in range(BPC)
    ]

    ctx = contextlib.ExitStack()
    with ctx:
        xts = [
            ctx.enter_context(nc.sbuf_tensor(f"xt{i}", [128, CW], f16))
            for i in range(BPC * NCH)
        ]
        ones16 = ctx.enter_context(nc.sbuf_tensor("ones16", [128, 1], f16))
        one_f = ctx.enter_context(nc.sbuf_tensor("one_f", [1, 1], f32))
        cst_sb = ctx.enter_context(nc.sbuf_tensor("cst_sb", [128, 384], f32))
        srows = [
            ctx.enter_context(nc.sbuf_tensor(f"srow{b}", [1, 2 * FEAT], f32))
            for b in range(BPC)
        ]
        stq = ctx.enter_context(nc.sbuf_tensor("stq", [128, 2 * BPC], f32))
        # per-batch padded result rows; col 128*b holds the real output
        res = ctx.enter_context(nc.sbuf_tensor("res", [128, BPC * 128], f32))

        pss = [
            ctx.enter_context(nc.psum_tensor(f"ps{b}", [1, 2 * FEAT], f32))
            for b in range(BPC)
        ]
        pts = ctx.enter_context(nc.psum_tensor("pts", [128, 2 * BPC], f32))
        pso = ctx.enter_context(nc.psum_tensor("pso", [128, BPC], f32))

        dsems = [
            ctx.enter_context(nc.semaphore(f"dsem{i}"))
            for i in range(BPC * NCH)
        ]
        csem = ctx.enter_context(nc.semaphore("csem"))    # const DMAs
        msem = ctx.enter_context(nc.semaphore("msem"))    # ones/one_f memsets
        pesem = ctx.enter_context(nc.semaphore("pesem"))  # PE milestones
        asem = ctx.enter_context(nc.semaphore("asem"))    # ACT srow copies
        vsem = ctx.enter_context(nc.semaphore("vsem"))    # DVE stq copies
        osem = ctx.enter_context(nc.semaphore("osem"))    # output DMAs
        sem_nums = sorted(
            s.num for s in (*dsems, csem, msem, pesem, asem, vsem, osem)
        )

        def xt(b, j):
            return xts[b * NCH + j]

        with nc.Block() as block:

            @block.sync
            def _(sync):
                # X chunk stream in consumption order on the SP HWDGE ring
                for b in range(BPC):
                    Xb = X_d[b].rearrange("(p r) f -> p (r f)", p=128)
                    for j in range(NCH):
                        sync.dma_start(
                            out=xt(b, j)[:, :],
                            in_=Xb[:, j * CW : (j + 1) * CW],
                        ).then_inc(dsems[b * NCH + j], 16)

            @block.scalar
            def _(scalar):
                # one line-rate const DMA on the ACT ring
                scalar.dma_start(out=cst_sb[:, :], in_=cst_d[:, :]).then_inc(csem, 16)
                scalar.wait_ge(csem, 16)
                # per-batch finale: srow copy, relu, padded output DMA
                for b in range(BPC):
                    scalar.wait_ge(pesem, 3 * b + 1)
                    nc.scalar.activation(
                        out=srows[b][:, :],
                        in_=pss[b][0:1, :],
                        func=AF.Copy,
                        scale=1.0,
                    ).then_inc(asem, 1)
                    scalar.wait_ge(pesem, 3 * b + 3)
                    nc.scalar.activation(
                        out=res[:, 128 * b : 128 * b + 1],
                        in_=pso[:, b : b + 1],
                        func=AF.Relu,
                        bias=cst_sb[:, 256:257],
                        scale=1.0,
                    )
                    scalar.dma_start(
                        out=outs_d[b][:, :], in_=res[:, 128 * b : 128 * (b + 1)]
                    ).then_inc(osem, 16)
                scalar.wait_ge(osem, 16 * BPC)

            @block.tensor
            def _(pe):
                pe.wait_ge(msem, 1)
                nmm = NCH * (CW // 512)
                for b in range(BPC):
                    # column-sum straight off the fp16 chunks
                    k = 0
                    lastc = None
                    for j in range(NCH):
                        pe.wait_ge(dsems[b * NCH + j], 16)
                        for m in range(CW // 512):
                            k += 1
                            lastc = nc.tensor.matmul(
                                pss[b][:, :],
                                lhsT=ones16[:, 0:1],
                                rhs=xt(b, j)[:, (m * 512) : (m + 1) * 512],
                                start=(k == 1),
                                stop=(k == nmm),
                            )
                    lastc.then_inc(pesem, 1)  # 3b+1
                    # fold even/odd halves + transpose into pts columns
                    pe.wait_ge(asem, b + 1)
                    last = None
                    for h in range(2):
                        nc.tensor.matmul(
                            pts[:, 2 * b + h : 2 * b + h + 1],
                            lhsT=srows[b][0:1, h * 128 : (h + 1) * 128],
                            rhs=one_f[0:1, 0:1],
                            is_transpose=True,
                            start=True,
                            stop=False,
                        )
                        last = nc.tensor.matmul(
                            pts[:, 2 * b + h : 2 * b + h + 1],
                            lhsT=srows[b][0:1, 256 + h * 128 : 256 + (h + 1) * 128],
                            rhs=one_f[0:1, 0:1],
                            is_transpose=True,
                            start=False,
                            stop=True,
                        )
                    last.then_inc(pesem, 1)  # 3b+2
                    # out_col[b] = W2^T @ s_feat[b]
                    pe.wait_ge(vsem, b + 1)
                    if b == 0:
                        pe.wait_ge(csem, 16)
                    nc.tensor.matmul(
                        pso[:, b : b + 1],
                        lhsT=cst_sb[:, 0:MEM],
                        rhs=stq[:, 2 * b : 2 * b + 1],
                        start=True,
                        stop=False,
                    )
                    nc.tensor.matmul(
                        pso[:, b : b + 1],
                        lhsT=cst_sb[:, MEM : 2 * MEM],
                        rhs=stq[:, 2 * b + 1 : 2 * b + 2],
                        start=False,
                        stop=True,
                    ).then_inc(pesem, 1)  # 3b+3

            @block.vector
            def _(vector):
                nc.vector.memset(ones16[:, :], 1.0)
                nc.vector.memset(one_f[:, :], 1.0)
                # zero the output padding (the padded out-DMA reads it; the
                # relu write is ordered after this via msem->...->pesem)
                nc.vector.memset(res[:, :], 0.0).then_inc(msem, 1)
                # psum transpose columns -> SBUF for the final matmul rhs
                for b in range(BPC):
                    vector.wait_ge(pesem, 3 * b + 2)
                    nc.vector.tensor_copy(
                        out=stq[:, 2 * b : 2 * b + 2], in_=pts[:, 2 * b : 2 * b + 2]
                    ).then_inc(vsem, 1)

            @block.gpsimd
            def _(gpsimd):
                gpsimd.wait_ge(osem, 16 * BPC)

            nc.all_engine_barrier()
            nc.gpsimd.sem_clear(range(sem_nums[0], sem_nums[-1] + 1))

    if not nc.is_finalized():
        nc.finalize()
    return nc


def kernel(X, mem, Wk, bk, Wv, bv, Wo, bo):
    global _built
    _ensure_axon_hooks()
    from concourse.bass_utils import run_bass_kernel_spmd

    if _built is None:
        _built = _build()
    nc = _built

    X16 = np.asarray(X).astype(np.float16)
    W2 = (
        (np.asarray(Wv, dtype=np.float64) / float(U))
        @ np.asarray(Wo, dtype=np.float64)
    ).astype(np.float32)
    c2 = (
        np.asarray(bv, dtype=np.float64) * (N / float(U))
    ) @ np.asarray(Wo, dtype=np.float64) + np.asarray(bo, dtype=np.float64)
    consts = np.zeros((128, 384), dtype=np.float32)
    consts[:, 0:128] = W2[0:128]
    consts[:, 128:256] = W2[128:256]
    consts[:, 256] = c2.astype(np.float32)

    in_maps = [
        {
            "Xs": np.ascontiguousarray(X16[i * BPC : (i + 1) * BPC]),
            "consts": consts,
        }
        for i in range(NCORES)
    ]
    r = run_bass_kernel_spmd(nc, in_maps, list(range(NCORES)))
    kernel._last_results = r

    out = np.empty((B, MEM), dtype=np.float32)
    for i in range(NCORES):
        for b in range(BPC):
            out[i * BPC + b] = r.results[i][f"out{b}"][:, 0]
    return out


# revision 31
# speedup vs baseline: 2.6401x; 1.0184x over previous
"""Trainium2 Bass kernel for nn_MemoryBlock (scatter_memory).

Mathematical identity: softmax over the memory-unit axis U produces rows
that sum to exactly 1, so

    out[b] = relu( mean_u( sum_n attn[b,n,u] * V[b,n,:] ) @ Wo + bo )
           = relu( (sum_n X[b,n,:]) @ W2 + c2 )

with W2 = (Wv/U) @ Wo and c2 = (N/U)*bv @ Wo + bo folded on the host --
the whole K/scores/softmax path cancels algebraically, leaving a
memory-bound column-sum of X plus one tiny matmul.

Device-side choices (per core, data-parallel over batch B):
- X is downcast to fp16 on the host (loose tolerance; colsum error is
  ~2e-4), halving HBM traffic to 8.4 MB/core.  The fp16 ones-matmul
  streams 2 cols/cycle, so TensorE consumes the raw chunks directly
  (no pre-reduction needed to keep up with the DMA stream).
- DMAs are strictly 128-partition: measured, non-128 partition counts
  fall off the DGE's optimized engine swizzle and halve the per-engine
  SDMA rate.  Layout is the classic [128 partitions x 64 rows]/batch.
- 16-row chunks = 8 KB per-partition descriptors (4 KB descriptors
  measured ~40% slower per engine).
- Finale per batch: ACT copies the colsum row from PSUM, PE transposes
  it into columns folding the even/odd interleave, one W2 matmul, relu
  with the folded bias, and ACT issues that batch's 512B output DMA
  immediately -- batch 0's output completes mid-stream, only batch 1's
  chain is in the tail.
"""

import contextlib

import numpy as np

B, N, FEAT, MEM, U = 16, 8192, 256, 128, 512
NCORES = 8
BPC = B // NCORES

RPP = N // 128     # 64 rows per partition per batch
# chunk row counts: big 8KB-descriptor chunks, then a small tail so the
# TensorE work left after the last byte lands is a single matmul
CHROWS = [16, 16, 16, 14, 2]
NCH = len(CHROWS)
CWS = [r * FEAT for r in CHROWS]          # fp16 cols per chunk
COFF = [sum(CWS[:j]) for j in range(NCH)]  # col offsets within a batch

_built = None


def _ensure_axon_hooks():
    try:
        import antenv.axon_hooks  # noqa: F401
        return
    except ImportError:
        pass
    import sys
    import types

    m = types.ModuleType("antenv.axon_hooks")
    holder = [None]
    m.set_axon_ntff_profile_hook = lambda h: holder.__setitem__(0, h)
    m.get_axon_ntff_profile_hook = lambda: holder[0]
    sys.modules["antenv.axon_hooks"] = m
    try:
        import antenv

        antenv.axon_hooks = m
    except ImportError:
        pass


def _build():
    import concourse.bacc as bacc
    import concourse.mybir as mybir

    f32 = mybir.dt.float32
    f16 = mybir.dt.float16
    AF = mybir.ActivationFunctionType
    nc = bacc.Bacc(None, enable_partition_id=False, monotonic_sem_count=0)

    X_d = nc.dram_tensor("Xs", [BPC, N, FEAT], f16, kind="ExternalInput")
    # all consts in one line-rate DMA: cols 0:128 = W2 half0, 128:256 =
    # W2 half1, col 256 = c2, padded to 384 cols (1.5KB/partition)
    cst_d = nc.dram_tensor("consts", [128, 384], f32, kind="ExternalInput")
    # outputs padded to 512B/partition: sub-512B HBM writes RMW and the
    # write receipt (which gates the DMA's semaphore) takes 3-7us
    outs_d = [
        nc.dram_tensor(f"out{b}", [MEM, 128], f32, kind="ExternalOutput")
        for b in range(BPC)
    ]

    ctx = contextlib.ExitStack()
    with ctx:
        xts = [
            ctx.enter_context(
                nc.sbuf_tensor(f"xt{b}_{j}", [128, CWS[j]], f16)
            )
            for b in range(BPC)
            for j in range(NCH)
        ]
        ones16 = ctx.enter_context(nc.sbuf_tensor("ones16", [128, 1], f16))
        one_f = ctx.enter_context(nc.sbuf_tensor("one_f", [1, 1], f32))
        cst_sb = ctx.enter_context(nc.sbuf_tensor("cst_sb", [128, 384], f32))
        srows = [
            ctx.enter_context(nc.sbuf_tensor(f"srow{b}", [1, 2 * FEAT], f32))
            for b in range(BPC)
        ]
        stq = ctx.enter_context(nc.sbuf_tensor("stq", [128, 2 * BPC], f32))
        # per-batch padded result rows; col 128*b holds the real output
        res = ctx.enter_context(nc.sbuf_tensor("res", [128, BPC * 128], f32))

        pss = [
            ctx.enter_context(nc.psum_tensor(f"ps{b}", [1, 2 * FEAT], f32))
            for b in range(BPC)
        ]
        pts = ctx.enter_context(nc.psum_tensor("pts", [128, 2 * BPC], f32))
        pso = ctx.enter_context(nc.psum_tensor("pso", [128, BPC], f32))

        dsems = [
            ctx.enter_context(nc.semaphore(f"dsem{i}"))
            for i in range(BPC * NCH)
        ]
        csem = ctx.enter_context(nc.semaphore("csem"))    # const DMAs
        msem = ctx.enter_context(nc.semaphore("msem"))    # ones/one_f memsets
        pesem = ctx.enter_context(nc.semaphore("pesem"))  # PE milestones
        asem = ctx.enter_context(nc.semaphore("asem"))    # ACT srow copies
        vsem = ctx.enter_context(nc.semaphore("vsem"))    # DVE stq copies
        osem = ctx.enter_context(nc.semaphore("osem"))    # output DMAs
        sem_nums = sorted(
            s.num for s in (*dsems, csem, msem, pesem, asem, vsem, osem)
        )

        def xt(b, j):
            return xts[b * NCH + j]

        with nc.Block() as block:

            @block.sync
            def _(sync):
                # X chunk stream in consumption order on the SP HWDGE ring
                for b in range(BPC):
                    Xb = X_d[b].rearrange("(p r) f -> p (r f)", p=128)
                    for j in range(NCH):
                        sync.dma_start(
                            out=xt(b, j)[:, :],
                            in_=Xb[:, COFF[j] : COFF[j] + CWS[j]],
                        ).then_inc(dsems[b * NCH + j], 16)

            @block.scalar
            def _(scalar):
                # one line-rate const DMA on the ACT ring
                scalar.dma_start(out=cst_sb[:, :], in_=cst_d[:, :]).then_inc(csem, 16)
                scalar.wait_ge(csem, 16)
                # pesem milestones: b0-colsum(1), b1-colsum(2), b0-T(3),
                # b0-W2(4), b1-T(5), b1-W2(6)
                for b in range(BPC):
                    scalar.wait_ge(pesem, b + 1)
                    nc.scalar.activation(
                        out=srows[b][:, :],
                        in_=pss[b][0:1, :],
                        func=AF.Copy,
                        scale=1.0,
                    ).then_inc(asem, 1)
                for b in range(BPC):
                    scalar.wait_ge(pesem, 2 * b + 4)
                    nc.scalar.activation(
                        out=res[:, 128 * b : 128 * b + 1],
                        in_=pso[:, b : b + 1],
                        func=AF.Relu,
                        bias=cst_sb[:, 256:257],
                        scale=1.0,
                    )
                    scalar.dma_start(
                        out=outs_d[b][:, :], in_=res[:, 128 * b : 128 * (b + 1)]
                    ).then_inc(osem, 16)
                scalar.wait_ge(osem, 16 * BPC)

            @block.tensor
            def _(pe):
                pe.wait_ge(msem, 1)
                # both column-sums back to back (the b0 finale's engine
                # round-trips must not delay b1's data consumption)
                for b in range(BPC):
                    k = 0
                    nmm = sum(CWS) // 512
                    lastc = None
                    for j in range(NCH):
                        pe.wait_ge(dsems[b * NCH + j], 16)
                        for m in range(CWS[j] // 512):
                            k += 1
                            lastc = nc.tensor.matmul(
                                pss[b][:, :],
                                lhsT=ones16[:, 0:1],
                                rhs=xt(b, j)[:, (m * 512) : (m + 1) * 512],
                                start=(k == 1),
                                stop=(k == nmm),
                            )
                    lastc.then_inc(pesem, 1)  # b+1
                for b in range(BPC):
                    # fold even/odd halves + transpose into pts columns
                    pe.wait_ge(asem, b + 1)
                    last = None
                    for h in range(2):
                        nc.tensor.matmul(
                            pts[:, 2 * b + h : 2 * b + h + 1],
                            lhsT=srows[b][0:1, h * 128 : (h + 1) * 128],
                            rhs=one_f[0:1, 0:1],
                            is_transpose=True,
                            start=True,
                            stop=False,
                        )
                        last = nc.tensor.matmul(
                            pts[:, 2 * b + h : 2 * b + h + 1],
                            lhsT=srows[b][0:1, 256 + h * 128 : 256 + (h + 1) * 128],
                            rhs=one_f[0:1, 0:1],
                            is_transpose=True,
                            start=False,
                            stop=True,
                        )
                    last.then_inc(pesem, 1)  # 2b+3
                    # out_col[b] = W2^T @ s_feat[b]
                    pe.wait_ge(vsem, b + 1)
                    if b == 0:
                        pe.wait_ge(csem, 16)
                    nc.tensor.matmul(
                        pso[:, b : b + 1],
                        lhsT=cst_sb[:, 0:MEM],
                        rhs=stq[:, 2 * b : 2 * b + 1],
                        start=True,
                        stop=False,
                    )
                    nc.tensor.matmul(
                        pso[:, b : b + 1],
                        lhsT=cst_sb[:, MEM : 2 * MEM],
                        rhs=stq[:, 2 * b + 1 : 2 * b + 2],
                        start=False,
                        stop=True,
                    ).then_inc(pesem, 1)  # 2b+4

            @block.vector
            def _(vector):
                nc.vector.memset(ones16[:, :], 1.0)
                nc.vector.memset(one_f[:, :], 1.0)
                # zero the output padding (the padded out-DMA reads it; the
                # relu write is ordered after this via msem->...->pesem)
                nc.vector.memset(res[:, :], 0.0).then_inc(msem, 1)
                # psum transpose columns -> SBUF for the final matmul rhs
                for b in range(BPC):
                    vector.wait_ge(pesem, 2 * b + 3)
                    nc.vector.tensor_copy(
                        out=stq[:, 2 * b : 2 * b + 2], in_=pts[:, 2 * b : 2 * b + 2]
                    ).then_inc(vsem, 1)

            @block.gpsimd
            def _(gpsimd):
                gpsimd.wait_ge(osem, 16 * BPC)

            nc.all_engine_barrier()
            nc.gpsimd.sem_clear(range(sem_nums[0], sem_nums[-1] + 1))

    if not nc.is_finalized():
        nc.finalize()
    return nc


def kernel(X, mem, Wk, bk, Wv, bv, Wo, bo):
    global _built
    _ensure_axon_hooks()
    from concourse.bass_utils import run_bass_kernel_spmd

    if _built is None:
        _built = _build()
    nc = _built

    X16 = np.asarray(X).astype(np.float16)
    W2 = (
        (np.asarray(Wv, dtype=np.float64) / float(U))
        @ np.asarray(Wo, dtype=np.float64)
    ).astype(np.float32)
    c2 = (
        np.asarray(bv, dtype=np.float64) * (N / float(U))
    ) @ np.asarray(Wo, dtype=np.float64) + np.asarray(bo, dtype=np.float64)
    consts = np.zeros((128, 384), dtype=np.float32)
    consts[:, 0:128] = W2[0:128]
    consts[:, 128:256] = W2[128:256]
    consts[:, 256] = c2.astype(np.float32)

    in_maps = [
        {
            "Xs": np.ascontiguousarray(X16[i * BPC : (i + 1) * BPC]),
            "consts": consts,
        }
        for i in range(NCORES)
    ]
    r = run_bass_kernel_spmd(nc, in_maps, list(range(NCORES)))
    kernel._last_results = r

    out = np.empty((B, MEM), dtype=np.float32)
    for i in range(NCORES):
        for b in range(BPC):
            out[i * BPC + b] = r.results[i][f"out{b}"][:, 0]
    return out
